# revision 5
# baseline (speedup 1.0000x reference)
"""Cross-threshold attention kernel for Trainium2 (8 NeuronCores, data parallel).

Computes, per batch b:
    q = x_src @ Wq^T + bq ;  k = x_tgt @ Wk^T + bk ;  v = x_tgt @ Wv^T + bv
    A = softmax(q @ k^T / sqrt(8))          [1024, 1024]  (written to HBM, fp32)
    out = x_src + A @ v                     [1024, 8]
for both directions (gpcm->coral and coral->gpcm).

Per-core layout (8 batches/core):
  - x^T [8, 1024] tiles produced by PE transposes; projections via tiny f32r
    matmuls with contraction over the threshold dim (K=8).
  - S row-chunks [128, 1024] on PE; exp on ACT with accum_out giving the
    softmax row-sums for free; normalize A = P * (1/s) on GPSIMD; A DMA'd out.
  - S^T chunks on PE, exp'd straight to bf16 P^T; A@v as 8 accumulated bf16
    matmuls (lhsT = P^T block, rhs = v chunk) into one PSUM bank; epilogue
    applies 1/s and the residual on DVE.
"""

import math

import numpy as np

import concourse.bacc as bacc
import concourse.tile as tile
from concourse import masks, mybir
from concourse.bass_utils import run_bass_kernel_spmd

F32 = mybir.dt.float32
F32R = mybir.dt.float32r
BF16 = mybir.dt.bfloat16

B, L, T = 64, 1024, 8
N_CORES = 8
NB = B // N_CORES          # batches per core
CH = 128                   # rows per chunk
NCH = L // CH              # chunks per batch
INV_SCALE = 1.0 / math.sqrt(T)

# S-matmul precision: "f32r" (fast, ~1e-4 rel err) or "f32" (slow, exact).
S_MODE = "f32r"
RDT = F32R if S_MODE == "f32r" else F32
# Engine for the A = P * r normalize pass: "gpsimd" or "vector".
NORM_ENGINE = "gpsimd"

_CACHE = {}


def _build():
    nc = bacc.Bacc("TRN2", target_bir_lowering=False, debug=False)

    xg = nc.dram_tensor("xg", [NB, L, T], F32, kind="ExternalInput")
    xc = nc.dram_tensor("xc", [NB, L, T], F32, kind="ExternalInput")
    wnames = ["Wq_g", "Wk_c", "Wv_c", "Wq_c", "Wk_g", "Wv_g"]
    bnames = ["bq_g", "bk_c", "bv_c", "bq_c", "bk_g", "bv_g"]
    wdram = {n: nc.dram_tensor(n, [T, T], F32, kind="ExternalInput") for n in wnames}
    bdram = {n: nc.dram_tensor(n, [T], F32, kind="ExternalInput") for n in bnames}

    attn_g = nc.dram_tensor("attn_g", [NB, L, L], F32, kind="ExternalOutput")
    attn_c = nc.dram_tensor("attn_c", [NB, L, L], F32, kind="ExternalOutput")
    out_g = nc.dram_tensor("out_g", [NB, L, T], F32, kind="ExternalOutput")
    out_c = nc.dram_tensor("out_c", [NB, L, T], F32, kind="ExternalOutput")

    with tile.TileContext(nc) as tc:
        with (
            tc.tile_pool(name="const", bufs=1) as const,
            tc.tile_pool(name="sb_small", bufs=2) as sb_small,
            tc.tile_pool(name="sb_p", bufs=3) as sb_p,
            tc.tile_pool(name="sb_a", bufs=3) as sb_a,
            tc.tile_pool(name="sb_pt", bufs=2) as sb_pt,
            tc.tile_pool(name="ps_big", bufs=2, space="PSUM") as ps_big,
            tc.tile_pool(name="ps_qk", bufs=2, space="PSUM") as ps_qk,
            tc.tile_pool(name="ps_acc", bufs=2, space="PSUM") as ps_acc,
        ):
            # ---- preamble: constants ----
            ident = const.tile([CH, CH], F32, tag="ident")
            masks.make_identity(nc, ident[:])

            wT = {}     # W^T [t, u] tiles in RDT
            for n in wnames:
                w_f32 = const.tile([T, T], F32, tag=f"{n}_f32")
                src = wdram[n].ap()
                src_t = type(src)(tensor=src.tensor, offset=0, ap=[[1, T], [T, T]])
                nc.sync.dma_start(out=w_f32[:], in_=src_t)
                if RDT is F32:
                    wT[n] = w_f32
                else:
                    w_r = const.tile([T, T], RDT, tag=f"{n}_rdt")
                    nc.vector.tensor_copy(w_r[:], w_f32[:])
                    wT[n] = w_r
            bcol = {}   # biases as [T, 1] columns
            for n in bnames:
                b_sb = const.tile([T, 1], F32, tag=f"{n}_col")
                src = bdram[n].ap()
                src2 = type(src)(tensor=src.tensor, offset=0, ap=[[1, T], [0, 1]])
                nc.sync.dma_start(out=b_sb[:], in_=src2)
                bcol[n] = b_sb

            directions = [
                # (x_src, x_tgt, Wq, bq, Wk, bk, Wv, bv, attn_out, out_out)
                ("g", "c", "Wq_g", "bq_g", "Wk_c", "bk_c", "Wv_c", "bv_c",
                 attn_g, out_g),
                ("c", "g", "Wq_c", "bq_c", "Wk_g", "bk_g", "Wv_g", "bv_g",
                 attn_c, out_c),
            ]
            xdram = {"g": xg, "c": xc}

            for bi in range(NB):
                # ---- load x chunks + build x^T for both tensors ----
                x_sb = {}
                xT_sb = {}
                for key in ("g", "c"):
                    xt_chunks = sb_small.tile([CH, NCH, T], F32, tag=f"x_{key}")
                    nc.sync.dma_start(
                        out=xt_chunks[:],
                        in_=xdram[key][bi].rearrange("(c p) t -> p c t", p=CH),
                    )
                    x_sb[key] = xt_chunks
                    xT = sb_small.tile([T, L], RDT, tag=f"xT_{key}")
                    for h in range(2):
                        xt_ps = ps_qk.tile([T, 512], F32, tag="qk")
                        for cc in range(4):
                            ic = h * 4 + cc
                            nc.tensor.transpose(
                                xt_ps[:, cc * CH:(cc + 1) * CH],
                                x_sb[key][:, ic, :],
                                ident[:],
                            )
                        nc.vector.tensor_copy(xT[:, h * 512:(h + 1) * 512], xt_ps[:])
                    xT_sb[key] = xT

                for (sk, tk, wq, bq, wk, bk, wv, bv, attn_out, out_out) in directions:
                    # ---- projections ----
                    qT = sb_small.tile([T, L], RDT, tag="qT")
                    kT = sb_small.tile([T, L], RDT, tag="kT")
                    for (dst, w, b, xsrc) in (
                        (qT, wq, bq, xT_sb[sk]),
                        (kT, wk, bk, xT_sb[tk]),
                    ):
                        for h in range(2):
                            pp = ps_qk.tile([T, 512], F32, tag="qk")
                            nc.tensor.matmul(
                                pp[:], wT[w][:], xsrc[:, h * 512:(h + 1) * 512]
                            )
                            nc.vector.tensor_scalar_add(
                                dst[:, h * 512:(h + 1) * 512], pp[:], bcol[b][:]
                            )
                    # v^T like q/k (whole-bank matmuls), then transpose to
                    # [j, u] layout (PE transposes don't reset the bank).
                    vT = sb_small.tile([T, L], F32, tag="vT")
                    for h in range(2):
                        pp = ps_qk.tile([T, 512], F32, tag="qk")
                        nc.tensor.matmul(
                            pp[:], wT[wv][:], xT_sb[tk][:, h * 512:(h + 1) * 512]
                        )
                        nc.vector.tensor_scalar_add(
                            vT[:, h * 512:(h + 1) * 512], pp[:], bcol[bv][:]
                        )
                    v_sb = sb_small.tile([CH, NCH, T], BF16, tag="v")
                    v_ps = ps_qk.tile([CH, NCH * T], F32, tag="qk")
                    for jc in range(NCH):
                        nc.tensor.transpose(
                            v_ps[:, jc * T:(jc + 1) * T],
                            vT[:, jc * CH:(jc + 1) * CH],
                            ident[0:T, 0:T],
                        )
                    nc.vector.tensor_copy(
                        v_sb[:].rearrange("p c t -> p (c t)"), v_ps[:]
                    )

                    # ---- S path: A = exp(S/sqrt(T)) / rowsum ----
                    s_sb = sb_small.tile([CH, NCH], F32, tag="s")
                    r_sb = sb_small.tile([CH, NCH], F32, tag="r")
                    for ic in range(NCH):
                        s_ps = ps_big.tile([CH, L], F32, tag="big")
                        for h in range(2):
                            nc.tensor.matmul(
                                s_ps[:, h * 512:(h + 1) * 512],
                                qT[:, ic * CH:(ic + 1) * CH],
                                kT[:, h * 512:(h + 1) * 512],
                            )
                        p_sb = sb_p.tile([CH, L], F32, tag="p")
                        nc.scalar.activation(
                            p_sb[:], s_ps[:], mybir.ActivationFunctionType.Exp,
                            scale=INV_SCALE, accum_out=s_sb[:, ic:ic + 1],
                        )
                        nc.vector.reciprocal(r_sb[:, ic:ic + 1], s_sb[:, ic:ic + 1])
                        a_sb = sb_a.tile([CH, L], F32, tag="a")
                        if NORM_ENGINE == "gpsimd":
                            nc.gpsimd.tensor_scalar_mul(
                                a_sb[:], p_sb[:], r_sb[:, ic:ic + 1]
                            )
                        else:
                            nc.vector.tensor_scalar_mul(
                                a_sb[:], p_sb[:], r_sb[:, ic:ic + 1]
                            )
                        nc.sync.dma_start(
                            out=attn_out[bi, ic * CH:(ic + 1) * CH, :], in_=a_sb[:]
                        )

                    # ---- S^T path: P^T (bf16) ----
                    pt_sb = sb_pt.tile([CH, NCH, L], BF16, tag="pt")
                    for jc in range(NCH):
                        st_ps = ps_big.tile([CH, L], F32, tag="big")
                        for h in range(2):
                            nc.tensor.matmul(
                                st_ps[:, h * 512:(h + 1) * 512],
                                kT[:, jc * CH:(jc + 1) * CH],
                                qT[:, h * 512:(h + 1) * 512],
                            )
                        nc.scalar.activation(
                            pt_sb[:, jc, :], st_ps[:],
                            mybir.ActivationFunctionType.Exp, scale=INV_SCALE,
                        )

                    # ---- A@v + epilogue (one PSUM tile per row-block: a
                    # matmul with start=True resets its whole bank) ----
                    fin = sb_small.tile([CH, NCH, T], F32, tag="fin")
                    for ib in range(NCH):
                        acc_ps = ps_acc.tile([CH, T], F32, tag="acc")
                        for jc in range(NCH):
                            nc.tensor.matmul(
                                acc_ps[:],
                                pt_sb[:, jc, ib * CH:(ib + 1) * CH],
                                v_sb[:, jc, :],
                                start=(jc == 0),
                                stop=(jc == NCH - 1),
                            )
                        nc.vector.scalar_tensor_tensor(
                            out=fin[:, ib, :],
                            in0=acc_ps[:],
                            scalar=r_sb[:, ib:ib + 1],
                            in1=x_sb[sk][:, ib, :],
                            op0=mybir.AluOpType.mult,
                            op1=mybir.AluOpType.add,
                        )
                    nc.sync.dma_start(
                        out=out_out[bi].rearrange("(c p) t -> p c t", p=CH),
                        in_=fin[:],
                    )

    nc.compile()
    return nc


def _get_nc():
    if "nc" not in _CACHE:
        _CACHE["nc"] = _build()
    return _CACHE["nc"]


def run_sharded(inputs, trace=False, **kw):
    """Run the SPMD kernel on 8 cores; returns (per-core results, BassKernelResults)."""
    nc = _get_nc()
    np_in = {k: np.asarray(v, dtype=np.float32) for k, v in inputs.items()}
    in_maps = []
    for c in range(N_CORES):
        m = {"xg": np_in["gpcm_thresholds"][c * NB:(c + 1) * NB],
             "xc": np_in["coral_thresholds"][c * NB:(c + 1) * NB]}
        for n in ("Wq_g", "Wk_c", "Wv_c", "Wq_c", "Wk_g", "Wv_g",
                  "bq_g", "bk_c", "bv_c", "bq_c", "bk_g", "bv_g"):
            m[n] = np_in[n]
        in_maps.append(m)
    res = run_bass_kernel_spmd(nc, in_maps, core_ids=list(range(N_CORES)),
                               trace=trace, **kw)
    return res


def kernel(**inputs):
    res = run_sharded(inputs)
    outs = res.results
    gpcm_att = np.concatenate([outs[c]["out_g"] for c in range(N_CORES)], axis=0)
    coral_att = np.concatenate([outs[c]["out_c"] for c in range(N_CORES)], axis=0)
    a_g2c = np.concatenate([outs[c]["attn_g"] for c in range(N_CORES)], axis=0)
    a_c2g = np.concatenate([outs[c]["attn_c"] for c in range(N_CORES)], axis=0)
    return (gpcm_att, coral_att,
            {"gpcm_to_coral": a_g2c, "coral_to_gpcm": a_c2g})


# revision 6
# speedup vs baseline: 4.2753x; 4.2753x over previous
"""Cross-threshold attention kernel for Trainium2 (8 NeuronCores, data parallel).

Computes, per batch b:
    q = x_src @ Wq^T + bq ;  k = x_tgt @ Wk^T + bk ;  v = x_tgt @ Wv^T + bv
    A = softmax(q @ k^T / sqrt(8))          [1024, 1024]  (written to HBM, fp32)
    out = x_src + A @ v                     [1024, 8]
for both directions (gpcm->coral and coral->gpcm).

Per-core layout (8 batches/core):
  - x^T [8, 1024] tiles produced by PE transposes; projections via tiny f32r
    matmuls with contraction over the threshold dim (K=8).
  - S row-chunks [128, 1024] on PE; exp on ACT with accum_out giving the
    softmax row-sums for free; normalize A = P * (1/s) on GPSIMD; A DMA'd out.
  - S^T chunks on PE, exp'd straight to bf16 P^T; A@v as 8 accumulated bf16
    matmuls (lhsT = P^T block, rhs = v chunk) into one PSUM bank; epilogue
    applies 1/s and the residual on DVE.
"""

import math

import numpy as np

import concourse.bacc as bacc
import concourse.tile as tile
from concourse import masks, mybir
from concourse.bass_utils import run_bass_kernel_spmd

F32 = mybir.dt.float32
F32R = mybir.dt.float32r
BF16 = mybir.dt.bfloat16

B, L, T = 64, 1024, 8
N_CORES = 8
NB = B // N_CORES          # batches per core
CH = 128                   # rows per chunk
NCH = L // CH              # chunks per batch
INV_SCALE = 1.0 / math.sqrt(T)

# S-matmul precision: "f32r" (fast, ~1e-4 rel err) or "f32" (slow, exact).
S_MODE = "f32r"
RDT = F32R if S_MODE == "f32r" else F32
# Engine for the A = P * r normalize pass: "gpsimd" or "vector".
NORM_ENGINE = "vector"

_CACHE = {}


def _build():
    nc = bacc.Bacc("TRN2", target_bir_lowering=False, debug=False)

    xg = nc.dram_tensor("xg", [NB, L, T], F32, kind="ExternalInput")
    xc = nc.dram_tensor("xc", [NB, L, T], F32, kind="ExternalInput")
    wnames = ["Wq_g", "Wk_c", "Wv_c", "Wq_c", "Wk_g", "Wv_g"]
    bnames = ["bq_g", "bk_c", "bv_c", "bq_c", "bk_g", "bv_g"]
    wdram = {n: nc.dram_tensor(n, [T, T], F32, kind="ExternalInput") for n in wnames}
    bdram = {n: nc.dram_tensor(n, [T], F32, kind="ExternalInput") for n in bnames}

    attn_g = nc.dram_tensor("attn_g", [NB, L, L], F32, kind="ExternalOutput")
    attn_c = nc.dram_tensor("attn_c", [NB, L, L], F32, kind="ExternalOutput")
    out_g = nc.dram_tensor("out_g", [NB, L, T], F32, kind="ExternalOutput")
    out_c = nc.dram_tensor("out_c", [NB, L, T], F32, kind="ExternalOutput")

    with tile.TileContext(nc) as tc:
        with (
            tc.tile_pool(name="const", bufs=1) as const,
            tc.tile_pool(name="sb_small", bufs=2) as sb_small,
            tc.tile_pool(name="sb_p", bufs=3) as sb_p,
            tc.tile_pool(name="sb_a", bufs=3) as sb_a,
            tc.tile_pool(name="sb_pt", bufs=2) as sb_pt,
            tc.tile_pool(name="ps_big", bufs=2, space="PSUM") as ps_big,
            tc.tile_pool(name="ps_qk", bufs=2, space="PSUM") as ps_qk,
            tc.tile_pool(name="ps_acc", bufs=2, space="PSUM") as ps_acc,
        ):
            # ---- preamble: constants ----
            ident = const.tile([CH, CH], F32, tag="ident")
            masks.make_identity(nc, ident[:])

            wT = {}     # W^T [t, u] tiles in RDT
            for n in wnames:
                w_f32 = const.tile([T, T], F32, tag=f"{n}_f32")
                src = wdram[n].ap()
                src_t = type(src)(tensor=src.tensor, offset=0, ap=[[1, T], [T, T]])
                nc.sync.dma_start(out=w_f32[:], in_=src_t)
                if RDT is F32:
                    wT[n] = w_f32
                else:
                    w_r = const.tile([T, T], RDT, tag=f"{n}_rdt")
                    nc.vector.tensor_copy(w_r[:], w_f32[:])
                    wT[n] = w_r
            bcol = {}   # biases as [T, 1] columns
            for n in bnames:
                b_sb = const.tile([T, 1], F32, tag=f"{n}_col")
                src = bdram[n].ap()
                src2 = type(src)(tensor=src.tensor, offset=0, ap=[[1, T], [0, 1]])
                nc.sync.dma_start(out=b_sb[:], in_=src2)
                bcol[n] = b_sb

            directions = [
                # (x_src, x_tgt, Wq, bq, Wk, bk, Wv, bv, attn_out, out_out)
                ("g", "c", "Wq_g", "bq_g", "Wk_c", "bk_c", "Wv_c", "bv_c",
                 attn_g, out_g),
                ("c", "g", "Wq_c", "bq_c", "Wk_g", "bk_g", "Wv_g", "bv_g",
                 attn_c, out_c),
            ]
            xdram = {"g": xg, "c": xc}

            for bi in range(NB):
                # ---- load x chunks + build x^T for both tensors ----
                x_sb = {}
                xT_sb = {}
                for key in ("g", "c"):
                    xt_chunks = sb_small.tile([CH, NCH, T], F32, tag=f"x_{key}")
                    nc.sync.dma_start(
                        out=xt_chunks[:],
                        in_=xdram[key][bi].rearrange("(c p) t -> p c t", p=CH),
                    )
                    x_sb[key] = xt_chunks
                    xT = sb_small.tile([T, L], RDT, tag=f"xT_{key}")
                    for h in range(2):
                        xt_ps = ps_qk.tile([T, 512], F32, tag="qk")
                        for cc in range(4):
                            ic = h * 4 + cc
                            nc.tensor.transpose(
                                xt_ps[:, cc * CH:(cc + 1) * CH],
                                x_sb[key][:, ic, :],
                                ident[:],
                            )
                        nc.vector.tensor_copy(xT[:, h * 512:(h + 1) * 512], xt_ps[:])
                    xT_sb[key] = xT

                for (sk, tk, wq, bq, wk, bk, wv, bv, attn_out, out_out) in directions:
                    # ---- projections ----
                    qT = sb_small.tile([T, L], RDT, tag="qT")
                    kT = sb_small.tile([T, L], RDT, tag="kT")
                    for (dst, w, b, xsrc) in (
                        (qT, wq, bq, xT_sb[sk]),
                        (kT, wk, bk, xT_sb[tk]),
                    ):
                        for h in range(2):
                            pp = ps_qk.tile([T, 512], F32, tag="qk")
                            nc.tensor.matmul(
                                pp[:], wT[w][:], xsrc[:, h * 512:(h + 1) * 512]
                            )
                            nc.vector.tensor_scalar_add(
                                dst[:, h * 512:(h + 1) * 512], pp[:], bcol[b][:]
                            )
                    # v^T like q/k (whole-bank matmuls), then transpose to
                    # [j, u] layout (PE transposes don't reset the bank).
                    vT = sb_small.tile([T, L], F32, tag="vT")
                    for h in range(2):
                        pp = ps_qk.tile([T, 512], F32, tag="qk")
                        nc.tensor.matmul(
                            pp[:], wT[wv][:], xT_sb[tk][:, h * 512:(h + 1) * 512]
                        )
                        nc.vector.tensor_scalar_add(
                            vT[:, h * 512:(h + 1) * 512], pp[:], bcol[bv][:]
                        )
                    v_sb = sb_small.tile([CH, NCH, T], BF16, tag="v")
                    v_ps = ps_qk.tile([CH, NCH * T], F32, tag="qk")
                    for jc in range(NCH):
                        nc.tensor.transpose(
                            v_ps[:, jc * T:(jc + 1) * T],
                            vT[:, jc * CH:(jc + 1) * CH],
                            ident[0:T, 0:T],
                        )
                    nc.vector.tensor_copy(
                        v_sb[:].rearrange("p c t -> p (c t)"), v_ps[:]
                    )

                    # ---- S path: A = exp(S/sqrt(T)) / rowsum ----
                    s_sb = sb_small.tile([CH, NCH], F32, tag="s")
                    r_sb = sb_small.tile([CH, NCH], F32, tag="r")
                    for ic in range(NCH):
                        s_ps = ps_big.tile([CH, L], F32, tag="big")
                        for h in range(2):
                            nc.tensor.matmul(
                                s_ps[:, h * 512:(h + 1) * 512],
                                qT[:, ic * CH:(ic + 1) * CH],
                                kT[:, h * 512:(h + 1) * 512],
                            )
                        p_sb = sb_p.tile([CH, L], F32, tag="p")
                        nc.scalar.activation(
                            p_sb[:], s_ps[:], mybir.ActivationFunctionType.Exp,
                            scale=INV_SCALE, accum_out=s_sb[:, ic:ic + 1],
                        )
                        nc.vector.reciprocal(r_sb[:, ic:ic + 1], s_sb[:, ic:ic + 1])
                        a_sb = sb_a.tile([CH, L], F32, tag="a")
                        if NORM_ENGINE == "gpsimd":
                            nc.gpsimd.tensor_scalar_mul(
                                a_sb[:], p_sb[:], r_sb[:, ic:ic + 1]
                            )
                        else:
                            nc.vector.tensor_scalar_mul(
                                a_sb[:], p_sb[:], r_sb[:, ic:ic + 1]
                            )
                        nc.sync.dma_start(
                            out=attn_out[bi, ic * CH:(ic + 1) * CH, :], in_=a_sb[:]
                        )

                    # ---- S^T path: P^T (bf16) ----
                    pt_sb = sb_pt.tile([CH, NCH, L], BF16, tag="pt")
                    for jc in range(NCH):
                        st_ps = ps_big.tile([CH, L], F32, tag="big")
                        for h in range(2):
                            nc.tensor.matmul(
                                st_ps[:, h * 512:(h + 1) * 512],
                                kT[:, jc * CH:(jc + 1) * CH],
                                qT[:, h * 512:(h + 1) * 512],
                            )
                        nc.scalar.activation(
                            pt_sb[:, jc, :], st_ps[:],
                            mybir.ActivationFunctionType.Exp, scale=INV_SCALE,
                        )

                    # ---- A@v + epilogue (one PSUM tile per row-block: a
                    # matmul with start=True resets its whole bank) ----
                    fin = sb_small.tile([CH, NCH, T], F32, tag="fin")
                    for ib in range(NCH):
                        acc_ps = ps_acc.tile([CH, T], F32, tag="acc")
                        for jc in range(NCH):
                            nc.tensor.matmul(
                                acc_ps[:],
                                pt_sb[:, jc, ib * CH:(ib + 1) * CH],
                                v_sb[:, jc, :],
                                start=(jc == 0),
                                stop=(jc == NCH - 1),
                            )
                        nc.vector.scalar_tensor_tensor(
                            out=fin[:, ib, :],
                            in0=acc_ps[:],
                            scalar=r_sb[:, ib:ib + 1],
                            in1=x_sb[sk][:, ib, :],
                            op0=mybir.AluOpType.mult,
                            op1=mybir.AluOpType.add,
                        )
                    nc.sync.dma_start(
                        out=out_out[bi].rearrange("(c p) t -> p c t", p=CH),
                        in_=fin[:],
                    )

    nc.compile()
    return nc


def _get_nc():
    if "nc" not in _CACHE:
        _CACHE["nc"] = _build()
    return _CACHE["nc"]


def run_sharded(inputs, trace=False, **kw):
    """Run the SPMD kernel on 8 cores; returns (per-core results, BassKernelResults)."""
    nc = _get_nc()
    np_in = {k: np.asarray(v, dtype=np.float32) for k, v in inputs.items()}
    in_maps = []
    for c in range(N_CORES):
        m = {"xg": np_in["gpcm_thresholds"][c * NB:(c + 1) * NB],
             "xc": np_in["coral_thresholds"][c * NB:(c + 1) * NB]}
        for n in ("Wq_g", "Wk_c", "Wv_c", "Wq_c", "Wk_g", "Wv_g",
                  "bq_g", "bk_c", "bv_c", "bq_c", "bk_g", "bv_g"):
            m[n] = np_in[n]
        in_maps.append(m)
    res = run_bass_kernel_spmd(nc, in_maps, core_ids=list(range(N_CORES)),
                               trace=trace, **kw)
    return res


def kernel(**inputs):
    res = run_sharded(inputs)
    outs = res.results
    gpcm_att = np.concatenate([outs[c]["out_g"] for c in range(N_CORES)], axis=0)
    coral_att = np.concatenate([outs[c]["out_c"] for c in range(N_CORES)], axis=0)
    a_g2c = np.concatenate([outs[c]["attn_g"] for c in range(N_CORES)], axis=0)
    a_c2g = np.concatenate([outs[c]["attn_c"] for c in range(N_CORES)], axis=0)
    return (gpcm_att, coral_att,
            {"gpcm_to_coral": a_g2c, "coral_to_gpcm": a_c2g})
